# revision 22
# baseline (speedup 1.0000x reference)
"""Fused attention block (QKV proj + softmax(QK^T/sqrt(d))V) for Trainium2,
SPMD over 8 NeuronCores.

Sharding: 8 shards = 4 batches x 2 sequence halves (keys column-rolled so
each core's own query block is rows 0:SH -- softmax + AV are permutation-
invariant over keys, so one uniform SPMD program serves all cores).

VERSION 3 (default): reassociated, collective-free, zero duplicated work.
Neither K nor V is ever materialized:
  scores^T = x @ (wk @ (wq^T x_q^T))   K-side:  QT -> G -> scores
  out      = ((x^T @ est)^T @ wv)/den  V-side:  AT -> out
Per-core matmul work is 15.05 GF = the ideal dedup floor (what v2 needs
pair-AllGathers for), ~191 us PE roofline at 78.6 TF/s bf16.

VERSION 1/2 kept for reference: v1 duplicates K/V projections per pair
(19.3 GF/core, ~290 us measured); v2 dedups them via pair-AllGather
(~261 us measured); v3 measured ~233 us.
"""

import time as time_mod
from contextlib import ExitStack

import numpy as np
import ml_dtypes

import concourse.bacc as bacc
import concourse.tile as tile
from concourse import mybir
from concourse.bass_utils import run_bass_kernel_spmd

B, S, D, E = 4, 2048, 1024, 1024  # batch, seq, model dim, qkv dim
SH = S // 2                       # per-core query rows
P = 128
DT = D // P   # 8 d-tiles (contraction for projections)
ET = E // P   # 8 e-tiles
JT = S // P   # 16 key tiles
IT = SH // P  # 8 query tiles
BF16 = mybir.dt.bfloat16
F32 = mybir.dt.float32
NPBF16 = ml_dtypes.bfloat16

_compiled = {}


NARROW_PSUM = True  # 512-wide psum groups, bufs=6 (more groups in flight)
EARLY_V = False     # emit V(s<8) between QT and KT to widen the DMA ramp


def _emit(tc, ctx, xTr, wq, wk, wv, out, rep=0, phases=("proj", "scores", "av")):
    nc = tc.nc
    CW = 512 if NARROW_PSUM else SH  # psum accumulation-group width
    const = ctx.enter_context(tc.tile_pool(name=f"const{rep}", bufs=1))
    mid = ctx.enter_context(tc.tile_pool(name=f"mid{rep}", bufs=1))
    psum = ctx.enter_context(tc.tile_pool(
        name=f"psum{rep}", bufs=(8 if NARROW_PSUM else 3), space="PSUM"))
    denp = psum if NARROW_PSUM else ctx.enter_context(
        tc.tile_pool(name=f"denp{rep}", bufs=2, space="PSUM"))
    outp = ctx.enter_context(tc.tile_pool(name=f"outp{rep}", bufs=3))
    small = ctx.enter_context(tc.tile_pool(name=f"small{rep}", bufs=4))

    ones = const.tile([P, 1], BF16, tag="ones")
    nc.vector.memset(ones, 1.0)

    # PE warm-up during the initial DMA fill: dummy matmuls on a zeroed tile
    # keep the clock-gate (HAM) warm so the real stream starts at full rate.
    warm_src = const.tile([P, 512], BF16, tag="warm_src")
    nc.vector.memset(warm_src, 0.0)
    warm_ps = psum.tile([P, 512], F32, tag="mm", name="warm_ps")
    for _ in range(4):
        nc.tensor.matmul(warm_ps, warm_src[:, 0:P], warm_src)

    with tc.tile_pool(name=f"ins{rep}", bufs=1) as ins:
        x_sb = [ins.tile([P, S], BF16, tag=f"x{d}", name=f"x{d}") for d in range(DT)]
        wq_sb = [ins.tile([P, E], BF16, tag=f"wq{d}", name=f"wq{d}") for d in range(DT)]
        wk_sb = [ins.tile([P, E], BF16, tag=f"wk{d}", name=f"wk{d}") for d in range(DT)]
        wv_sb = [ins.tile([P, E], BF16, tag=f"wv{d}", name=f"wv{d}") for d in range(DT)]
        # Load order tracks first-use: QT needs wq + x[:, :SH]; KT adds wk +
        # x[:, SH:]; V needs wv last. Splitting the x DMA lets QT matmuls
        # start after ~4MB instead of ~10MB.
        for d in range(DT):
            r = slice(d * P, (d + 1) * P)
            nc.sync.dma_start(wq_sb[d], wq[r, :])
            nc.sync.dma_start(x_sb[d][:, 0:SH], xTr[r, 0:SH])
        if EARLY_V:
            for d in range(DT):
                nc.sync.dma_start(wv_sb[d], wv[d * P:(d + 1) * P, :])
            for d in range(DT):
                r = slice(d * P, (d + 1) * P)
                nc.sync.dma_start(wk_sb[d], wk[r, :])
                nc.sync.dma_start(x_sb[d][:, SH:S], xTr[r, SH:S])
        else:
            for d in range(DT):
                r = slice(d * P, (d + 1) * P)
                nc.sync.dma_start(wk_sb[d], wk[r, :])
                nc.sync.dma_start(x_sb[d][:, SH:S], xTr[r, SH:S])
            for d in range(DT):
                nc.sync.dma_start(wv_sb[d], wv[d * P:(d + 1) * P, :])

        qt_sb = [mid.tile([P, SH], BF16, tag=f"qt{e}", name=f"qt{e}") for e in range(ET)]
        kt_sb = [mid.tile([P, S], BF16, tag=f"kt{e}", name=f"kt{e}") for e in range(ET)]
        v_sb = [mid.tile([P, E], BF16, tag=f"v{s}", name=f"v{s}") for s in range(JT)]

        # QT[e,:] = sum_d wq[d, e-block].T @ xTr[d, :SH]
        for e in range(ET if "proj" in phases else 0):
            for c0 in range(0, SH, CW):
                ps = psum.tile([P, CW], F32, tag="mm", name="mm_ps")
                for d in range(DT):
                    lhsT = wq_sb[d][:, e * P:(e + 1) * P]
                    for h in range(c0, c0 + CW, 512):
                        nc.tensor.matmul(ps[:, h - c0:h - c0 + 512], lhsT,
                                         x_sb[d][:, h:h + 512],
                                         start=(d == 0), stop=(d == DT - 1))
                nc.vector.tensor_copy(qt_sb[e][:, c0:c0 + CW], ps)

        # V first half (s<IT needs only x[:, :SH] + wv) can fill the ramp
        if EARLY_V:
            for s in range(IT if "proj" in phases else 0):
                for c0 in range(0, E, CW):
                    ps = psum.tile([P, CW], F32, tag="mm", name="mm_ps")
                    for d in range(DT):
                        lhsT = x_sb[d][:, s * P:(s + 1) * P]
                        for h in range(c0, c0 + CW, 512):
                            nc.tensor.matmul(ps[:, h - c0:h - c0 + 512], lhsT,
                                             wv_sb[d][:, h:h + 512],
                                             start=(d == 0), stop=(d == DT - 1))
                    nc.scalar.copy(v_sb[s][:, c0:c0 + CW], ps)

        # KT[e,:] = sum_d wk[d, e-block].T @ xTr[d, :]
        for e in range(ET if "proj" in phases else 0):
            for c0 in range(0, S, CW):
                ps = psum.tile([P, CW], F32, tag="mm", name="mm_ps")
                for d in range(DT):
                    lhsT = wk_sb[d][:, e * P:(e + 1) * P]
                    for h in range(c0, c0 + CW, 512):
                        nc.tensor.matmul(ps[:, h - c0:h - c0 + 512], lhsT,
                                         x_sb[d][:, h:h + 512],
                                         start=(d == 0), stop=(d == DT - 1))
                nc.vector.tensor_copy(kt_sb[e][:, c0:c0 + CW], ps)

        # V[s,:] = sum_d xTr[d, s-block].T @ wv[d, :]
        for s in range(IT if EARLY_V else 0, JT if "proj" in phases else 0):
            for c0 in range(0, E, CW):
                ps = psum.tile([P, CW], F32, tag="mm", name="mm_ps")
                for d in range(DT):
                    lhsT = x_sb[d][:, s * P:(s + 1) * P]
                    for h in range(c0, c0 + CW, 512):
                        nc.tensor.matmul(ps[:, h - c0:h - c0 + 512], lhsT,
                                         wv_sb[d][:, h:h + 512],
                                         start=(d == 0), stop=(d == DT - 1))
                nc.scalar.copy(v_sb[s][:, c0:c0 + CW], ps)

        # scores^T[j-block, :] = sum_e KT[e, j-block].T @ QT[e, :]; exp fused
        est_sb = [mid.tile([P, SH], BF16, tag=f"est{j}", name=f"est{j}") for j in range(JT)]
        for j in range(JT if "scores" in phases else 0):
            for c0 in range(0, SH, CW):
                ps = psum.tile([P, CW], F32, tag="mm", name="mm_ps")
                for e in range(ET):
                    lhsT = kt_sb[e][:, j * P:(j + 1) * P]
                    for h in range(c0, c0 + CW, 512):
                        nc.tensor.matmul(ps[:, h - c0:h - c0 + 512], lhsT,
                                         qt_sb[e][:, h:h + 512],
                                         start=(e == 0), stop=(e == ET - 1))
                # exp(scores / sqrt(E)); scores ~ N(0,1): no max subtraction
                nc.scalar.activation(est_sb[j][:, c0:c0 + CW], ps,
                                     mybir.ActivationFunctionType.Exp,
                                     scale=float(1.0 / np.sqrt(E)))

    # attn @ V, with the softmax denominator from a ones-matmul sharing lhsT
    for i in range(IT if "av" in phases else 0):
        den = denp.tile([P, 1], F32, tag=("mm" if NARROW_PSUM else "den"),
                        name="den_ps")
        avs = []
        for c0 in range(0, E, CW):
            av = psum.tile([P, CW], F32, tag="mm", name="av_ps")
            for j in range(JT):
                lhsT = est_sb[j][:, i * P:(i + 1) * P]
                for h in range(c0, c0 + CW, 512):
                    nc.tensor.matmul(av[:, h - c0:h - c0 + 512], lhsT,
                                     v_sb[j][:, h:h + 512],
                                     start=(j == 0), stop=(j == JT - 1))
                if c0 == 0:
                    nc.tensor.matmul(den, lhsT, ones,
                                     start=(j == 0), stop=(j == JT - 1))
            avs.append(av)
        recip = small.tile([P, 1], F32, tag="recip")
        nc.vector.reciprocal(recip, den)
        o = outp.tile([P, E], F32, tag="o")
        for ci, av in enumerate(avs):
            nc.vector.tensor_scalar_mul(o[:, ci * CW:(ci + 1) * CW], av, recip)
        nc.sync.dma_start(out[i * P:(i + 1) * P, :], o)


def _emit_v2(tc, ctx, xqT, wq, wk, wv, out, rep=0):
    """K/V-dedup variant: compute KT/V only for this core's own SH rows and
    pair-AllGather them (keys in batch order) while QT/scores run on PE."""
    nc = tc.nc
    groups = [[0, 1], [2, 3], [4, 5], [6, 7]]
    NH = SH // 512  # 512-wide chunks per SH

    const = ctx.enter_context(tc.tile_pool(name=f"c{rep}", bufs=1))
    mid = ctx.enter_context(tc.tile_pool(name=f"m{rep}", bufs=1))
    psum = ctx.enter_context(tc.tile_pool(name=f"p{rep}", bufs=3, space="PSUM"))
    denp = ctx.enter_context(tc.tile_pool(name=f"d{rep}", bufs=2, space="PSUM"))
    outp = ctx.enter_context(tc.tile_pool(name=f"o{rep}", bufs=3))
    small = ctx.enter_context(tc.tile_pool(name=f"s{rep}", bufs=4))
    dram = ctx.enter_context(tc.tile_pool(name=f"dr{rep}", bufs=1, space="DRAM"))

    ones = const.tile([P, 1], BF16, tag="ones")
    nc.vector.memset(ones, 1.0)

    # PE warm-up during the initial DMA fill (see _emit)
    warm_src = const.tile([P, 512], BF16, tag="warm_src")
    nc.vector.memset(warm_src, 0.0)
    warm_ps = psum.tile([P, 512], F32, tag="mm", name="warm_ps")
    for _ in range(4):
        nc.tensor.matmul(warm_ps, warm_src[:, 0:P], warm_src)

    ktg_in = dram.tile([E, SH], BF16, tag="ktg_in")
    ktg_out = dram.tile([2, E, SH], BF16, tag="ktg_out")
    vg_in = dram.tile([SH, E], BF16, tag="vg_in")
    vg_out = dram.tile([2, SH, E], BF16, tag="vg_out")

    kt_sb = [mid.tile([P, S], BF16, tag=f"kt{e}", name=f"kt{e}") for e in range(ET)]
    v_sb = [mid.tile([P, E], BF16, tag=f"v{s}", name=f"v{s}") for s in range(JT)]
    qt_sb = [mid.tile([P, SH], BF16, tag=f"qt{e}", name=f"qt{e}") for e in range(ET)]
    est_sb = [mid.tile([P, SH], BF16, tag=f"est{j}", name=f"est{j}")
              for j in range(JT)]

    with tc.tile_pool(name=f"i{rep}", bufs=1) as ins:
        xq_sb = [ins.tile([P, SH], BF16, tag=f"x{d}", name=f"x{d}")
                 for d in range(DT)]
        wq_sb = [ins.tile([P, E], BF16, tag=f"wq{d}", name=f"wq{d}")
                 for d in range(DT)]
        wk_sb = [ins.tile([P, E], BF16, tag=f"wk{d}", name=f"wk{d}")
                 for d in range(DT)]
        wv_sb = [ins.tile([P, E], BF16, tag=f"wv{d}", name=f"wv{d}")
                 for d in range(DT)]
        # load order tracks first use: KTo needs wk+xq, then wv (Vo), wq (QT)
        for d in range(DT):
            r = slice(d * P, (d + 1) * P)
            nc.sync.dma_start(wk_sb[d], wk[r, :])
            nc.sync.dma_start(xq_sb[d], xqT[r, :])
        for d in range(DT):
            r = slice(d * P, (d + 1) * P)
            nc.sync.dma_start(wv_sb[d], wv[r, :])
        for d in range(DT):
            r = slice(d * P, (d + 1) * P)
            nc.sync.dma_start(wq_sb[d], wq[r, :])

        # KTo[e,:] = wk[:, e-block].T @ xqT  -> bounce -> AllGather
        for e in range(ET):
            ps = psum.tile([P, SH], F32, tag="mm", name="mm_ps")
            for d in range(DT):
                lhsT = wk_sb[d][:, e * P:(e + 1) * P]
                for h in range(NH):
                    c = slice(h * 512, (h + 1) * 512)
                    nc.tensor.matmul(ps[:, c], lhsT, xq_sb[d][:, c],
                                     start=(d == 0), stop=(d == DT - 1))
            kto = outp.tile([P, SH], BF16, tag="kto", name="kto")
            nc.vector.tensor_copy(kto, ps)
            nc.sync.dma_start(ktg_in[e * P:(e + 1) * P, :], kto)
        nc.gpsimd.collective_compute(
            "AllGather", mybir.AluOpType.bypass, replica_groups=groups,
            ins=[ktg_in.opt()], outs=[ktg_out.opt()])

        # Vo[s,:] = xqT[:, s-block].T @ wv  -> bounce -> AllGather
        for s in range(IT):
            ps = psum.tile([P, SH], F32, tag="mm", name="mm_ps")
            for d in range(DT):
                lhsT = xq_sb[d][:, s * P:(s + 1) * P]
                for h in range(NH):
                    c = slice(h * 512, (h + 1) * 512)
                    nc.tensor.matmul(ps[:, c], lhsT, wv_sb[d][:, c],
                                     start=(d == 0), stop=(d == DT - 1))
            vo = outp.tile([P, E], BF16, tag="vo", name="vo")
            nc.scalar.copy(vo, ps)
            nc.sync.dma_start(vg_in[s * P:(s + 1) * P, :], vo)
        nc.gpsimd.collective_compute(
            "AllGather", mybir.AluOpType.bypass, replica_groups=groups,
            ins=[vg_in.opt()], outs=[vg_out.opt()])

        # QT[e,:] = wq[:, e-block].T @ xqT  (overlaps the collectives)
        for e in range(ET):
            ps = psum.tile([P, SH], F32, tag="mm", name="mm_ps")
            for d in range(DT):
                lhsT = wq_sb[d][:, e * P:(e + 1) * P]
                for h in range(NH):
                    c = slice(h * 512, (h + 1) * 512)
                    nc.tensor.matmul(ps[:, c], lhsT, xq_sb[d][:, c],
                                     start=(d == 0), stop=(d == DT - 1))
            nc.vector.tensor_copy(qt_sb[e], ps)

    # gathered KT back to SBUF (batch order: block r = pair-rank r's rows)
    for e in range(ET):
        for r in range(2):
            nc.sync.dma_start(kt_sb[e][:, r * SH:(r + 1) * SH],
                              ktg_out[r, e * P:(e + 1) * P, :])

    # scores^T + fused exp
    for j in range(JT):
        ps = psum.tile([P, SH], F32, tag="mm", name="mm_ps")
        for e in range(ET):
            lhsT = kt_sb[e][:, j * P:(j + 1) * P]
            for h in range(NH):
                c = slice(h * 512, (h + 1) * 512)
                nc.tensor.matmul(ps[:, c], lhsT, qt_sb[e][:, c],
                                 start=(e == 0), stop=(e == ET - 1))
        nc.scalar.activation(est_sb[j], ps, mybir.ActivationFunctionType.Exp,
                             scale=float(1.0 / np.sqrt(E)))

    # gathered V back to SBUF
    for s in range(JT):
        r, sl = s // IT, s % IT
        nc.sync.dma_start(v_sb[s], vg_out[r, sl * P:(sl + 1) * P, :])

    # attn @ V with ones-matmul denominator
    for i in range(IT):
        av = psum.tile([P, E], F32, tag="mm", name="av_ps")
        den = denp.tile([P, 1], F32, tag="den")
        for j in range(JT):
            lhsT = est_sb[j][:, i * P:(i + 1) * P]
            for h in range(E // 512):
                c = slice(h * 512, (h + 1) * 512)
                nc.tensor.matmul(av[:, c], lhsT, v_sb[j][:, c],
                                 start=(j == 0), stop=(j == JT - 1))
            nc.tensor.matmul(den, lhsT, ones,
                             start=(j == 0), stop=(j == JT - 1))
        recip = small.tile([P, 1], F32, tag="recip")
        nc.vector.reciprocal(recip, den)
        o = outp.tile([P, E], F32, tag="o", name="o_out")
        nc.vector.tensor_scalar_mul(o, av, recip)
        nc.sync.dma_start(out[i * P:(i + 1) * P, :], o)


DEN_MODE = "tiny"  # "tiny": 128 [P,1] MMs (ldw-heavy) | "row": 32 N=512 MMs
                  # with shared 1-col ones lhsT + DRAM-bounce transpose
WIDE_PSUM = True   # [P,1024] 2-bank psum groups; one ACT/DVE evacuation each
SC_MODE = "exp"    # "exp" | "copy" (timing probe) | "split" (DVE evac + ACT)
OUT_BF16 = True    # write the output tensor in bf16 (host upcasts)


def _emit_v3(tc, ctx, xTr, xN, wq, wkT, wv, out, rep=0,
             phases=("qt", "g", "sc", "at", "den", "out")):
    """Reassociated attention, collective-free, 15.05 GF/core (the dedup floor).

    scores^T = x @ (wk @ (wq^T x_q^T)) -- K never materialized
    out      = (x^T @ est)^T @ wv      -- V never materialized
    Every matmul keeps its contraction dim on partitions; per-core work is
    all intrinsic (no duplicated projections, no collectives):
      QT [E,SH] = wq.T @ xTr[:, :SH]          (128 MM)
      G  [D,SH] = wkT.T @ QT                  (128 MM)
      ST [S,SH] = xTr.T @ G ; est = exp(ST/32) (256 MM)
      AT [D,SH] = xN.T @ est                  (256 MM)
      den[SH,1] = est.T @ ones                (128 tiny MM)
      out[SH,E] = (AT.T @ wv) * (1/den)       (128 MM)
    """
    nc = tc.nc
    CW = 512
    const = ctx.enter_context(tc.tile_pool(name=f"c3_{rep}", bufs=1))
    mid = ctx.enter_context(tc.tile_pool(name=f"m3_{rep}", bufs=1))
    # PSUM: 8 banks total. wide mode: 4 x [P,1024] (2 banks each);
    # narrow: 8 x [P,512]. "row" den reserves 2 banks -> shrink the mm ring.
    mm_bufs = (3 if DEN_MODE == "row" else 4) if WIDE_PSUM else \
              (6 if DEN_MODE == "row" else 8)
    psum = ctx.enter_context(tc.tile_pool(
        name=f"p3_{rep}", bufs=mm_bufs, space="PSUM"))

    GW = 2 * CW if WIDE_PSUM else CW  # accumulation-group width

    def _groups(pool, width):
        for c0 in range(0, width, GW):
            cw = min(GW, width - c0)
            yield pool.tile([P, cw], F32, tag="mm", name="mm_ps"), c0, cw
    outp = ctx.enter_context(tc.tile_pool(name=f"o3_{rep}", bufs=3))
    small = ctx.enter_context(tc.tile_pool(name=f"s3_{rep}", bufs=4))
    late = ctx.enter_context(tc.tile_pool(name=f"l3_{rep}", bufs=1))

    ones = const.tile([P, 1], BF16, tag="ones")
    nc.vector.memset(ones, 1.0)

    # PE warm-up during the initial DMA fill (keeps HAM clock-gate warm)
    warm_src = const.tile([P, 512], BF16, tag="warm_src")
    nc.vector.memset(warm_src, 0.0)
    warm_ps = psum.tile([P, 512], F32, tag="mm", name="warm_ps")
    for _ in range(4):
        nc.tensor.matmul(warm_ps, warm_src[:, 0:P], warm_src)

    qt_sb = [mid.tile([P, SH], BF16, tag=f"qt{e}", name=f"qt{e}") for e in range(ET)]
    g_sb = [mid.tile([P, SH], BF16, tag=f"g{d}", name=f"g{d}") for d in range(DT)]
    est_sb = [mid.tile([P, SH], BF16, tag=f"est{j}", name=f"est{j}")
              for j in range(JT)]

    xn_sb = [late.tile([P, D], BF16, tag=f"xn{s}", name=f"xn{s}") for s in range(JT)]
    wv_sb = [late.tile([P, E], BF16, tag=f"wv{d}", name=f"wv{d}") for d in range(DT)]

    with tc.tile_pool(name=f"e3_{rep}", bufs=1) as early:
        xt_sb = [early.tile([P, S], BF16, tag=f"xt{d}", name=f"xt{d}")
                 for d in range(DT)]
        wq_sb = [early.tile([P, E], BF16, tag=f"wq{d}", name=f"wq{d}")
                 for d in range(DT)]
        wkT_sb = [early.tile([P, D], BF16, tag=f"wkT{e}", name=f"wkT{e}")
                  for e in range(ET)]
        # load order tracks first use: QT needs wq + xTr[:, :SH]; G adds wkT;
        # scoresT adds xTr[:, SH:]; AT needs xN; out needs wv last.
        for d in range(DT):
            r = slice(d * P, (d + 1) * P)
            nc.sync.dma_start(wq_sb[d], wq[r, :])
            nc.sync.dma_start(xt_sb[d][:, 0:SH], xTr[r, 0:SH])
        for e in range(ET):
            nc.sync.dma_start(wkT_sb[e], wkT[e * P:(e + 1) * P, :])
        for d in range(DT):
            nc.sync.dma_start(xt_sb[d][:, SH:S], xTr[d * P:(d + 1) * P, SH:S])
        for s in range(JT):
            nc.sync.dma_start(xn_sb[s], xN[s * P:(s + 1) * P, :])
        for d in range(DT):
            nc.sync.dma_start(wv_sb[d], wv[d * P:(d + 1) * P, :])

        # QT[e-block, :] = sum_d wq[d, e-block].T @ xTr[d, :SH]
        for e in range(ET if "qt" in phases else 0):
            for ps, c0, cw in _groups(psum, SH):
                for d in range(DT):
                    for h in range(0, cw, CW):
                        nc.tensor.matmul(ps[:, h:h + CW],
                                         wq_sb[d][:, e * P:(e + 1) * P],
                                         xt_sb[d][:, c0 + h:c0 + h + CW],
                                         start=(d == 0), stop=(d == DT - 1))
                nc.vector.tensor_copy(qt_sb[e][:, c0:c0 + cw], ps)

        # G[d-block, :] = sum_e wkT[e, d-block].T @ QT[e, :]
        for dblk in range(DT if "g" in phases else 0):
            for ps, c0, cw in _groups(psum, SH):
                for e in range(ET):
                    for h in range(0, cw, CW):
                        nc.tensor.matmul(ps[:, h:h + CW],
                                         wkT_sb[e][:, dblk * P:(dblk + 1) * P],
                                         qt_sb[e][:, c0 + h:c0 + h + CW],
                                         start=(e == 0), stop=(e == ET - 1))
                nc.vector.tensor_copy(g_sb[dblk][:, c0:c0 + cw], ps)

        # scores^T[j-block, :] = sum_d xTr[d, j-block].T @ G[d, :]; exp fused
        sctmp = [small.tile([P, SH], F32, tag="sctmp", bufs=2,
                            name=f"sctmp{t}") for t in range(2)] \
            if SC_MODE == "split" else None
        for j in range(JT if "sc" in phases else 0):
            for ps, c0, cw in _groups(psum, SH):
                for d in range(DT):
                    for h in range(0, cw, CW):
                        nc.tensor.matmul(ps[:, h:h + CW],
                                         xt_sb[d][:, j * P:(j + 1) * P],
                                         g_sb[d][:, c0 + h:c0 + h + CW],
                                         start=(d == 0), stop=(d == DT - 1))
                if SC_MODE == "none":  # timing probe: no evacuation
                    pass
                elif SC_MODE == "exp":
                    nc.scalar.activation(est_sb[j][:, c0:c0 + cw], ps,
                                         mybir.ActivationFunctionType.Exp,
                                         scale=float(1.0 / np.sqrt(E)))
                elif SC_MODE == "copy":  # timing probe only: wrong numerics
                    nc.vector.tensor_copy(est_sb[j][:, c0:c0 + cw], ps)
                else:  # "split": DVE evacuates PSUM, ACT exps from SBUF
                    tmp = sctmp[j % 2]
                    nc.vector.tensor_copy(tmp[:, c0:c0 + cw], ps)
                    nc.scalar.activation(est_sb[j][:, c0:c0 + cw],
                                         tmp[:, c0:c0 + cw],
                                         mybir.ActivationFunctionType.Exp,
                                         scale=float(1.0 / np.sqrt(E)))

    # AT[d-block, :] = sum_s xN[s, d-block].T @ est[s, :]
    atp = ctx.enter_context(tc.tile_pool(name=f"a3_{rep}", bufs=1))
    at_sb = [atp.tile([P, SH], BF16, tag=f"at{d}", name=f"at{d}")
             for d in range(DT)]
    for dblk in range(DT if "at" in phases else 0):
        for ps, c0, cw in _groups(psum, SH):
            for sj in range(JT):
                for h in range(0, cw, CW):
                    nc.tensor.matmul(ps[:, h:h + CW],
                                     xn_sb[sj][:, dblk * P:(dblk + 1) * P],
                                     est_sb[sj][:, c0 + h:c0 + h + CW],
                                     start=(sj == 0), stop=(sj == JT - 1))
            nc.vector.tensor_copy(at_sb[dblk][:, c0:c0 + cw], ps)

    # den[q] = sum_s est[s, q]
    recips = None
    recip_t = None
    if "den" in phases and DEN_MODE == "row":
        # ones-lhsT form: den_row[0, q] accumulated over all 16 s-tiles.
        # lhsT is 1 column -> weight load is free; 32 N=512 matmuls total.
        den_ps = psum.tile([P, SH], F32, tag="den", bufs=1, name="den_ps")
        for c0 in range(0, SH, CW):
            for j in range(JT):
                nc.tensor.matmul(den_ps[0:1, c0:c0 + CW], ones,
                                 est_sb[j][:, c0:c0 + CW],
                                 start=(j == 0), stop=(j == JT - 1))
        den_sb = small.tile([1, SH], F32, tag="den_sb", name="den_sb")
        nc.scalar.copy(den_sb, den_ps[0:1, :])
        dram = ctx.enter_context(tc.tile_pool(name=f"dr3_{rep}", bufs=1,
                                              space="DRAM"))
        dr_den = dram.tile([1, SH], F32, tag="dr_den", name="dr_den")
        nc.sync.dma_start(dr_den, den_sb)
        den_t = small.tile([P, IT], F32, tag="den_t", name="den_t")
        nc.sync.dma_start(den_t,
                          dr_den[0, :].rearrange("(i p) -> p i", p=P))
        recip_t = small.tile([P, IT], F32, tag="recip_t", name="recip_t")
        nc.vector.reciprocal(recip_t, den_t)
    elif "den" in phases:
        recips = []
        for i in range(IT):
            den = psum.tile([P, 1], F32, tag="mm", name="den_ps")
            for j in range(JT):
                nc.tensor.matmul(den, est_sb[j][:, i * P:(i + 1) * P], ones,
                                 start=(j == 0), stop=(j == JT - 1))
            recip = small.tile([P, 1], F32, tag="recip", bufs=IT, name="recip")
            nc.vector.reciprocal(recip, den)
            recips.append(recip)

    # out[i-block, :] = (sum_d AT[d, i-block].T @ wv[d, :]) * recip_i
    for i in range(IT if "out" in phases and "den" in phases else 0):
        o = outp.tile([P, E], BF16 if OUT_BF16 else F32, tag="o", name="o_out")
        r_i = recip_t[:, i:i + 1] if recip_t is not None else recips[i]
        for av, c0, cw in _groups(psum, E):
            for d in range(DT):
                for h in range(0, cw, CW):
                    nc.tensor.matmul(av[:, h:h + CW],
                                     at_sb[d][:, i * P:(i + 1) * P],
                                     wv_sb[d][:, c0 + h:c0 + h + CW],
                                     start=(d == 0), stop=(d == DT - 1))
            nc.vector.tensor_scalar_mul(o[:, c0:c0 + cw], av, r_i)
        nc.scalar.dma_start(out[i * P:(i + 1) * P, :], o)


def _build_v3(repeats=1, phases=("qt", "g", "sc", "at", "den", "out")):
    key = ("v3", repeats, tuple(phases), WIDE_PSUM, OUT_BF16, DEN_MODE,
           SC_MODE)
    if key not in _compiled:
        nc = bacc.Bacc("TRN2", target_bir_lowering=False, debug=False,
                       num_devices=8)
        xTr = nc.dram_tensor("xTr", [D, S], BF16, kind="ExternalInput").ap()
        xN = nc.dram_tensor("xN", [S, D], BF16, kind="ExternalInput").ap()
        wq = nc.dram_tensor("wq", [D, E], BF16, kind="ExternalInput").ap()
        wkT = nc.dram_tensor("wkT", [E, D], BF16, kind="ExternalInput").ap()
        wv = nc.dram_tensor("wv", [D, E], BF16, kind="ExternalInput").ap()
        out = nc.dram_tensor("out", [SH, E], BF16 if OUT_BF16 else F32,
                             kind="ExternalOutput").ap()
        with tile.TileContext(nc) as tc:
            for rep in range(repeats):
                with ExitStack() as ctx:
                    _emit_v3(tc, ctx, xTr, xN, wq, wkT, wv, out, rep=rep,
                             phases=phases)
        nc.compile()
        _compiled[key] = nc
    return _compiled[key]


def _make_in_maps_v3(x, wq, wk, wv):
    wq_bf = np.ascontiguousarray(wq).astype(NPBF16)
    wkT_bf = np.ascontiguousarray(np.asarray(wk).T).astype(NPBF16)
    wv_bf = np.ascontiguousarray(wv).astype(NPBF16)
    in_maps = []
    for c in range(8):
        b, h = c // 2, c % 2
        # roll keys so this core's query block is always rows 0:SH
        xr = np.concatenate([x[b, h * SH:(h + 1) * SH], x[b, :h * SH],
                             x[b, (h + 1) * SH:]], axis=0)
        xN = np.ascontiguousarray(xr).astype(NPBF16)
        xTr = np.ascontiguousarray(xr.T).astype(NPBF16)
        in_maps.append({"xTr": xTr, "xN": xN, "wq": wq_bf, "wkT": wkT_bf,
                        "wv": wv_bf})
    return in_maps


def _build_v2(repeats=1):
    key = ("v2", repeats)
    if key not in _compiled:
        nc = bacc.Bacc("TRN2", target_bir_lowering=False, debug=False,
                       num_devices=8)
        xqT = nc.dram_tensor("xqT", [D, SH], BF16, kind="ExternalInput").ap()
        wq = nc.dram_tensor("wq", [D, E], BF16, kind="ExternalInput").ap()
        wk = nc.dram_tensor("wk", [D, E], BF16, kind="ExternalInput").ap()
        wv = nc.dram_tensor("wv", [D, E], BF16, kind="ExternalInput").ap()
        out = nc.dram_tensor("out", [SH, E], F32, kind="ExternalOutput").ap()
        with tile.TileContext(nc) as tc:
            for rep in range(repeats):
                with ExitStack() as ctx:
                    _emit_v2(tc, ctx, xqT, wq, wk, wv, out, rep=rep)
        nc.compile()
        _compiled[key] = nc
    return _compiled[key]


def _make_in_maps_v2(x, wq, wk, wv):
    wq_bf = np.ascontiguousarray(wq).astype(NPBF16)
    wk_bf = np.ascontiguousarray(wk).astype(NPBF16)
    wv_bf = np.ascontiguousarray(wv).astype(NPBF16)
    in_maps = []
    for c in range(8):
        b, h = c // 2, c % 2
        xqT = np.ascontiguousarray(x[b, h * SH:(h + 1) * SH].T).astype(NPBF16)
        in_maps.append({"xqT": xqT, "wq": wq_bf, "wk": wk_bf, "wv": wv_bf})
    return in_maps


def _build(repeats=1, phases=("proj", "scores", "av")):
    key = (repeats, tuple(phases), NARROW_PSUM, EARLY_V)
    if key not in _compiled:
        nc = bacc.Bacc("TRN2", target_bir_lowering=False, debug=False,
                       num_devices=8)
        xTr = nc.dram_tensor("xTr", [D, S], BF16, kind="ExternalInput").ap()
        wq = nc.dram_tensor("wq", [D, E], BF16, kind="ExternalInput").ap()
        wk = nc.dram_tensor("wk", [D, E], BF16, kind="ExternalInput").ap()
        wv = nc.dram_tensor("wv", [D, E], BF16, kind="ExternalInput").ap()
        out = nc.dram_tensor("out", [SH, E], F32, kind="ExternalOutput").ap()
        with tile.TileContext(nc) as tc:
            for rep in range(repeats):
                with ExitStack() as ctx:
                    _emit(tc, ctx, xTr, wq, wk, wv, out, rep=rep, phases=phases)
        nc.compile()
        _compiled[key] = nc
    return _compiled[key]


def _make_in_maps(x, wq, wk, wv):
    wq_bf = np.ascontiguousarray(wq).astype(NPBF16)
    wk_bf = np.ascontiguousarray(wk).astype(NPBF16)
    wv_bf = np.ascontiguousarray(wv).astype(NPBF16)
    in_maps = []
    for c in range(8):
        b, h = c // 2, c % 2
        # roll keys so this core's query block is always columns 0:SH
        xr = np.concatenate([x[b, h * SH:(h + 1) * SH], x[b, :h * SH],
                             x[b, (h + 1) * SH:]], axis=0)
        xTr = np.ascontiguousarray(xr.T).astype(NPBF16)
        in_maps.append({"xTr": xTr, "wq": wq_bf, "wk": wk_bf, "wv": wv_bf})
    return in_maps


VERSION = 3


def _build_any(repeats=1):
    if VERSION == 3:
        return _build_v3(repeats)
    return _build_v2(repeats) if VERSION == 2 else _build(repeats)


def _make_maps_any(x, wq, wk, wv):
    mk = {1: _make_in_maps, 2: _make_in_maps_v2, 3: _make_in_maps_v3}[VERSION]
    return mk(np.asarray(x, np.float32), np.asarray(wq), np.asarray(wk),
              np.asarray(wv))


def kernel(x, wq, wk, wv, _trace=False):
    x = np.asarray(x, dtype=np.float32)
    nc = _build_any()
    in_maps = _make_maps_any(x, wq, wk, wv)
    try:
        res = run_bass_kernel_spmd(nc, in_maps, core_ids=list(range(8)),
                                   trace=_trace)
    except Exception:
        # transient NRT_EXEC_UNIT_UNRECOVERABLE wedges have been observed to
        # clear on a fresh attempt
        time_mod.sleep(5)
        res = run_bass_kernel_spmd(nc, in_maps, core_ids=list(range(8)),
                                   trace=_trace)
    full = np.empty((B, S, E), np.float32)
    for c in range(8):
        b, h = c // 2, c % 2
        full[b, h * SH:(h + 1) * SH] = np.asarray(
            res.results[c]["out"], dtype=np.float32)
    if _trace:
        kernel.last_results = res
    return full



# revision 23
# speedup vs baseline: 1.0045x; 1.0045x over previous
"""Fused attention block (QKV proj + softmax(QK^T/sqrt(d))V) for Trainium2,
SPMD over 8 NeuronCores.

Sharding: 8 shards = 4 batches x 2 sequence halves (keys column-rolled so
each core's own query block is rows 0:SH -- softmax + AV are permutation-
invariant over keys, so one uniform SPMD program serves all cores).

VERSION 3 (default): reassociated, collective-free, zero duplicated work.
Neither K nor V is ever materialized:
  scores^T = x @ (wk @ (wq^T x_q^T))   K-side:  QT -> G -> scores
  out      = ((x^T @ est)^T @ wv)/den  V-side:  AT -> out
Per-core matmul work is 15.05 GF = the ideal dedup floor (what v2 needs
pair-AllGathers for), ~191 us PE roofline at 78.6 TF/s bf16.

VERSION 1/2 kept for reference: v1 duplicates K/V projections per pair
(19.3 GF/core, ~290 us measured); v2 dedups them via pair-AllGather
(~261 us measured); v3 measured ~233 us.
"""

import time as time_mod
from contextlib import ExitStack

import numpy as np
import ml_dtypes

import concourse.bacc as bacc
import concourse.tile as tile
from concourse import mybir
from concourse.bass_utils import run_bass_kernel_spmd

B, S, D, E = 4, 2048, 1024, 1024  # batch, seq, model dim, qkv dim
SH = S // 2                       # per-core query rows
P = 128
DT = D // P   # 8 d-tiles (contraction for projections)
ET = E // P   # 8 e-tiles
JT = S // P   # 16 key tiles
IT = SH // P  # 8 query tiles
BF16 = mybir.dt.bfloat16
F32 = mybir.dt.float32
NPBF16 = ml_dtypes.bfloat16

_compiled = {}


NARROW_PSUM = True  # 512-wide psum groups, bufs=6 (more groups in flight)
EARLY_V = False     # emit V(s<8) between QT and KT to widen the DMA ramp


def _emit(tc, ctx, xTr, wq, wk, wv, out, rep=0, phases=("proj", "scores", "av")):
    nc = tc.nc
    CW = 512 if NARROW_PSUM else SH  # psum accumulation-group width
    const = ctx.enter_context(tc.tile_pool(name=f"const{rep}", bufs=1))
    mid = ctx.enter_context(tc.tile_pool(name=f"mid{rep}", bufs=1))
    psum = ctx.enter_context(tc.tile_pool(
        name=f"psum{rep}", bufs=(8 if NARROW_PSUM else 3), space="PSUM"))
    denp = psum if NARROW_PSUM else ctx.enter_context(
        tc.tile_pool(name=f"denp{rep}", bufs=2, space="PSUM"))
    outp = ctx.enter_context(tc.tile_pool(name=f"outp{rep}", bufs=3))
    small = ctx.enter_context(tc.tile_pool(name=f"small{rep}", bufs=4))

    ones = const.tile([P, 1], BF16, tag="ones")
    nc.vector.memset(ones, 1.0)

    # PE warm-up during the initial DMA fill: dummy matmuls on a zeroed tile
    # keep the clock-gate (HAM) warm so the real stream starts at full rate.
    warm_src = const.tile([P, 512], BF16, tag="warm_src")
    nc.vector.memset(warm_src, 0.0)
    warm_ps = psum.tile([P, 512], F32, tag="mm", name="warm_ps")
    for _ in range(4):
        nc.tensor.matmul(warm_ps, warm_src[:, 0:P], warm_src)

    with tc.tile_pool(name=f"ins{rep}", bufs=1) as ins:
        x_sb = [ins.tile([P, S], BF16, tag=f"x{d}", name=f"x{d}") for d in range(DT)]
        wq_sb = [ins.tile([P, E], BF16, tag=f"wq{d}", name=f"wq{d}") for d in range(DT)]
        wk_sb = [ins.tile([P, E], BF16, tag=f"wk{d}", name=f"wk{d}") for d in range(DT)]
        wv_sb = [ins.tile([P, E], BF16, tag=f"wv{d}", name=f"wv{d}") for d in range(DT)]
        # Load order tracks first-use: QT needs wq + x[:, :SH]; KT adds wk +
        # x[:, SH:]; V needs wv last. Splitting the x DMA lets QT matmuls
        # start after ~4MB instead of ~10MB.
        for d in range(DT):
            r = slice(d * P, (d + 1) * P)
            nc.sync.dma_start(wq_sb[d], wq[r, :])
            nc.sync.dma_start(x_sb[d][:, 0:SH], xTr[r, 0:SH])
        if EARLY_V:
            for d in range(DT):
                nc.sync.dma_start(wv_sb[d], wv[d * P:(d + 1) * P, :])
            for d in range(DT):
                r = slice(d * P, (d + 1) * P)
                nc.sync.dma_start(wk_sb[d], wk[r, :])
                nc.sync.dma_start(x_sb[d][:, SH:S], xTr[r, SH:S])
        else:
            for d in range(DT):
                r = slice(d * P, (d + 1) * P)
                nc.sync.dma_start(wk_sb[d], wk[r, :])
                nc.sync.dma_start(x_sb[d][:, SH:S], xTr[r, SH:S])
            for d in range(DT):
                nc.sync.dma_start(wv_sb[d], wv[d * P:(d + 1) * P, :])

        qt_sb = [mid.tile([P, SH], BF16, tag=f"qt{e}", name=f"qt{e}") for e in range(ET)]
        kt_sb = [mid.tile([P, S], BF16, tag=f"kt{e}", name=f"kt{e}") for e in range(ET)]
        v_sb = [mid.tile([P, E], BF16, tag=f"v{s}", name=f"v{s}") for s in range(JT)]

        # QT[e,:] = sum_d wq[d, e-block].T @ xTr[d, :SH]
        for e in range(ET if "proj" in phases else 0):
            for c0 in range(0, SH, CW):
                ps = psum.tile([P, CW], F32, tag="mm", name="mm_ps")
                for d in range(DT):
                    lhsT = wq_sb[d][:, e * P:(e + 1) * P]
                    for h in range(c0, c0 + CW, 512):
                        nc.tensor.matmul(ps[:, h - c0:h - c0 + 512], lhsT,
                                         x_sb[d][:, h:h + 512],
                                         start=(d == 0), stop=(d == DT - 1))
                nc.vector.tensor_copy(qt_sb[e][:, c0:c0 + CW], ps)

        # V first half (s<IT needs only x[:, :SH] + wv) can fill the ramp
        if EARLY_V:
            for s in range(IT if "proj" in phases else 0):
                for c0 in range(0, E, CW):
                    ps = psum.tile([P, CW], F32, tag="mm", name="mm_ps")
                    for d in range(DT):
                        lhsT = x_sb[d][:, s * P:(s + 1) * P]
                        for h in range(c0, c0 + CW, 512):
                            nc.tensor.matmul(ps[:, h - c0:h - c0 + 512], lhsT,
                                             wv_sb[d][:, h:h + 512],
                                             start=(d == 0), stop=(d == DT - 1))
                    nc.scalar.copy(v_sb[s][:, c0:c0 + CW], ps)

        # KT[e,:] = sum_d wk[d, e-block].T @ xTr[d, :]
        for e in range(ET if "proj" in phases else 0):
            for c0 in range(0, S, CW):
                ps = psum.tile([P, CW], F32, tag="mm", name="mm_ps")
                for d in range(DT):
                    lhsT = wk_sb[d][:, e * P:(e + 1) * P]
                    for h in range(c0, c0 + CW, 512):
                        nc.tensor.matmul(ps[:, h - c0:h - c0 + 512], lhsT,
                                         x_sb[d][:, h:h + 512],
                                         start=(d == 0), stop=(d == DT - 1))
                nc.vector.tensor_copy(kt_sb[e][:, c0:c0 + CW], ps)

        # V[s,:] = sum_d xTr[d, s-block].T @ wv[d, :]
        for s in range(IT if EARLY_V else 0, JT if "proj" in phases else 0):
            for c0 in range(0, E, CW):
                ps = psum.tile([P, CW], F32, tag="mm", name="mm_ps")
                for d in range(DT):
                    lhsT = x_sb[d][:, s * P:(s + 1) * P]
                    for h in range(c0, c0 + CW, 512):
                        nc.tensor.matmul(ps[:, h - c0:h - c0 + 512], lhsT,
                                         wv_sb[d][:, h:h + 512],
                                         start=(d == 0), stop=(d == DT - 1))
                nc.scalar.copy(v_sb[s][:, c0:c0 + CW], ps)

        # scores^T[j-block, :] = sum_e KT[e, j-block].T @ QT[e, :]; exp fused
        est_sb = [mid.tile([P, SH], BF16, tag=f"est{j}", name=f"est{j}") for j in range(JT)]
        for j in range(JT if "scores" in phases else 0):
            for c0 in range(0, SH, CW):
                ps = psum.tile([P, CW], F32, tag="mm", name="mm_ps")
                for e in range(ET):
                    lhsT = kt_sb[e][:, j * P:(j + 1) * P]
                    for h in range(c0, c0 + CW, 512):
                        nc.tensor.matmul(ps[:, h - c0:h - c0 + 512], lhsT,
                                         qt_sb[e][:, h:h + 512],
                                         start=(e == 0), stop=(e == ET - 1))
                # exp(scores / sqrt(E)); scores ~ N(0,1): no max subtraction
                nc.scalar.activation(est_sb[j][:, c0:c0 + CW], ps,
                                     mybir.ActivationFunctionType.Exp,
                                     scale=float(1.0 / np.sqrt(E)))

    # attn @ V, with the softmax denominator from a ones-matmul sharing lhsT
    for i in range(IT if "av" in phases else 0):
        den = denp.tile([P, 1], F32, tag=("mm" if NARROW_PSUM else "den"),
                        name="den_ps")
        avs = []
        for c0 in range(0, E, CW):
            av = psum.tile([P, CW], F32, tag="mm", name="av_ps")
            for j in range(JT):
                lhsT = est_sb[j][:, i * P:(i + 1) * P]
                for h in range(c0, c0 + CW, 512):
                    nc.tensor.matmul(av[:, h - c0:h - c0 + 512], lhsT,
                                     v_sb[j][:, h:h + 512],
                                     start=(j == 0), stop=(j == JT - 1))
                if c0 == 0:
                    nc.tensor.matmul(den, lhsT, ones,
                                     start=(j == 0), stop=(j == JT - 1))
            avs.append(av)
        recip = small.tile([P, 1], F32, tag="recip")
        nc.vector.reciprocal(recip, den)
        o = outp.tile([P, E], F32, tag="o")
        for ci, av in enumerate(avs):
            nc.vector.tensor_scalar_mul(o[:, ci * CW:(ci + 1) * CW], av, recip)
        nc.sync.dma_start(out[i * P:(i + 1) * P, :], o)


def _emit_v2(tc, ctx, xqT, wq, wk, wv, out, rep=0):
    """K/V-dedup variant: compute KT/V only for this core's own SH rows and
    pair-AllGather them (keys in batch order) while QT/scores run on PE."""
    nc = tc.nc
    groups = [[0, 1], [2, 3], [4, 5], [6, 7]]
    NH = SH // 512  # 512-wide chunks per SH

    const = ctx.enter_context(tc.tile_pool(name=f"c{rep}", bufs=1))
    mid = ctx.enter_context(tc.tile_pool(name=f"m{rep}", bufs=1))
    psum = ctx.enter_context(tc.tile_pool(name=f"p{rep}", bufs=3, space="PSUM"))
    denp = ctx.enter_context(tc.tile_pool(name=f"d{rep}", bufs=2, space="PSUM"))
    outp = ctx.enter_context(tc.tile_pool(name=f"o{rep}", bufs=3))
    small = ctx.enter_context(tc.tile_pool(name=f"s{rep}", bufs=4))
    dram = ctx.enter_context(tc.tile_pool(name=f"dr{rep}", bufs=1, space="DRAM"))

    ones = const.tile([P, 1], BF16, tag="ones")
    nc.vector.memset(ones, 1.0)

    # PE warm-up during the initial DMA fill (see _emit)
    warm_src = const.tile([P, 512], BF16, tag="warm_src")
    nc.vector.memset(warm_src, 0.0)
    warm_ps = psum.tile([P, 512], F32, tag="mm", name="warm_ps")
    for _ in range(4):
        nc.tensor.matmul(warm_ps, warm_src[:, 0:P], warm_src)

    ktg_in = dram.tile([E, SH], BF16, tag="ktg_in")
    ktg_out = dram.tile([2, E, SH], BF16, tag="ktg_out")
    vg_in = dram.tile([SH, E], BF16, tag="vg_in")
    vg_out = dram.tile([2, SH, E], BF16, tag="vg_out")

    kt_sb = [mid.tile([P, S], BF16, tag=f"kt{e}", name=f"kt{e}") for e in range(ET)]
    v_sb = [mid.tile([P, E], BF16, tag=f"v{s}", name=f"v{s}") for s in range(JT)]
    qt_sb = [mid.tile([P, SH], BF16, tag=f"qt{e}", name=f"qt{e}") for e in range(ET)]
    est_sb = [mid.tile([P, SH], BF16, tag=f"est{j}", name=f"est{j}")
              for j in range(JT)]

    with tc.tile_pool(name=f"i{rep}", bufs=1) as ins:
        xq_sb = [ins.tile([P, SH], BF16, tag=f"x{d}", name=f"x{d}")
                 for d in range(DT)]
        wq_sb = [ins.tile([P, E], BF16, tag=f"wq{d}", name=f"wq{d}")
                 for d in range(DT)]
        wk_sb = [ins.tile([P, E], BF16, tag=f"wk{d}", name=f"wk{d}")
                 for d in range(DT)]
        wv_sb = [ins.tile([P, E], BF16, tag=f"wv{d}", name=f"wv{d}")
                 for d in range(DT)]
        # load order tracks first use: KTo needs wk+xq, then wv (Vo), wq (QT)
        for d in range(DT):
            r = slice(d * P, (d + 1) * P)
            nc.sync.dma_start(wk_sb[d], wk[r, :])
            nc.sync.dma_start(xq_sb[d], xqT[r, :])
        for d in range(DT):
            r = slice(d * P, (d + 1) * P)
            nc.sync.dma_start(wv_sb[d], wv[r, :])
        for d in range(DT):
            r = slice(d * P, (d + 1) * P)
            nc.sync.dma_start(wq_sb[d], wq[r, :])

        # KTo[e,:] = wk[:, e-block].T @ xqT  -> bounce -> AllGather
        for e in range(ET):
            ps = psum.tile([P, SH], F32, tag="mm", name="mm_ps")
            for d in range(DT):
                lhsT = wk_sb[d][:, e * P:(e + 1) * P]
                for h in range(NH):
                    c = slice(h * 512, (h + 1) * 512)
                    nc.tensor.matmul(ps[:, c], lhsT, xq_sb[d][:, c],
                                     start=(d == 0), stop=(d == DT - 1))
            kto = outp.tile([P, SH], BF16, tag="kto", name="kto")
            nc.vector.tensor_copy(kto, ps)
            nc.sync.dma_start(ktg_in[e * P:(e + 1) * P, :], kto)
        nc.gpsimd.collective_compute(
            "AllGather", mybir.AluOpType.bypass, replica_groups=groups,
            ins=[ktg_in.opt()], outs=[ktg_out.opt()])

        # Vo[s,:] = xqT[:, s-block].T @ wv  -> bounce -> AllGather
        for s in range(IT):
            ps = psum.tile([P, SH], F32, tag="mm", name="mm_ps")
            for d in range(DT):
                lhsT = xq_sb[d][:, s * P:(s + 1) * P]
                for h in range(NH):
                    c = slice(h * 512, (h + 1) * 512)
                    nc.tensor.matmul(ps[:, c], lhsT, wv_sb[d][:, c],
                                     start=(d == 0), stop=(d == DT - 1))
            vo = outp.tile([P, E], BF16, tag="vo", name="vo")
            nc.scalar.copy(vo, ps)
            nc.sync.dma_start(vg_in[s * P:(s + 1) * P, :], vo)
        nc.gpsimd.collective_compute(
            "AllGather", mybir.AluOpType.bypass, replica_groups=groups,
            ins=[vg_in.opt()], outs=[vg_out.opt()])

        # QT[e,:] = wq[:, e-block].T @ xqT  (overlaps the collectives)
        for e in range(ET):
            ps = psum.tile([P, SH], F32, tag="mm", name="mm_ps")
            for d in range(DT):
                lhsT = wq_sb[d][:, e * P:(e + 1) * P]
                for h in range(NH):
                    c = slice(h * 512, (h + 1) * 512)
                    nc.tensor.matmul(ps[:, c], lhsT, xq_sb[d][:, c],
                                     start=(d == 0), stop=(d == DT - 1))
            nc.vector.tensor_copy(qt_sb[e], ps)

    # gathered KT back to SBUF (batch order: block r = pair-rank r's rows)
    for e in range(ET):
        for r in range(2):
            nc.sync.dma_start(kt_sb[e][:, r * SH:(r + 1) * SH],
                              ktg_out[r, e * P:(e + 1) * P, :])

    # scores^T + fused exp
    for j in range(JT):
        ps = psum.tile([P, SH], F32, tag="mm", name="mm_ps")
        for e in range(ET):
            lhsT = kt_sb[e][:, j * P:(j + 1) * P]
            for h in range(NH):
                c = slice(h * 512, (h + 1) * 512)
                nc.tensor.matmul(ps[:, c], lhsT, qt_sb[e][:, c],
                                 start=(e == 0), stop=(e == ET - 1))
        nc.scalar.activation(est_sb[j], ps, mybir.ActivationFunctionType.Exp,
                             scale=float(1.0 / np.sqrt(E)))

    # gathered V back to SBUF
    for s in range(JT):
        r, sl = s // IT, s % IT
        nc.sync.dma_start(v_sb[s], vg_out[r, sl * P:(sl + 1) * P, :])

    # attn @ V with ones-matmul denominator
    for i in range(IT):
        av = psum.tile([P, E], F32, tag="mm", name="av_ps")
        den = denp.tile([P, 1], F32, tag="den")
        for j in range(JT):
            lhsT = est_sb[j][:, i * P:(i + 1) * P]
            for h in range(E // 512):
                c = slice(h * 512, (h + 1) * 512)
                nc.tensor.matmul(av[:, c], lhsT, v_sb[j][:, c],
                                 start=(j == 0), stop=(j == JT - 1))
            nc.tensor.matmul(den, lhsT, ones,
                             start=(j == 0), stop=(j == JT - 1))
        recip = small.tile([P, 1], F32, tag="recip")
        nc.vector.reciprocal(recip, den)
        o = outp.tile([P, E], F32, tag="o", name="o_out")
        nc.vector.tensor_scalar_mul(o, av, recip)
        nc.sync.dma_start(out[i * P:(i + 1) * P, :], o)


DEN_MODE = "row"  # "tiny": 128 [P,1] MMs (ldw-heavy) | "row": 32 N=512 MMs
                  # with shared 1-col ones lhsT + DRAM-bounce transpose
WIDE_PSUM = True   # [P,1024] 2-bank psum groups; one ACT/DVE evacuation each
SC_MODE = "split"    # "exp" | "copy" (timing probe) | "split" (DVE evac + ACT)
OUT_BF16 = True    # write the output tensor in bf16 (host upcasts)


def _emit_v3(tc, ctx, xTr, xN, wq, wkT, wv, out, rep=0,
             phases=("qt", "g", "sc", "at", "den", "out")):
    """Reassociated attention, collective-free, 15.05 GF/core (the dedup floor).

    scores^T = x @ (wk @ (wq^T x_q^T)) -- K never materialized
    out      = (x^T @ est)^T @ wv      -- V never materialized
    Every matmul keeps its contraction dim on partitions; per-core work is
    all intrinsic (no duplicated projections, no collectives):
      QT [E,SH] = wq.T @ xTr[:, :SH]          (128 MM)
      G  [D,SH] = wkT.T @ QT                  (128 MM)
      ST [S,SH] = xTr.T @ G ; est = exp(ST/32) (256 MM)
      AT [D,SH] = xN.T @ est                  (256 MM)
      den[SH,1] = est.T @ ones                (128 tiny MM)
      out[SH,E] = (AT.T @ wv) * (1/den)       (128 MM)
    """
    nc = tc.nc
    CW = 512
    const = ctx.enter_context(tc.tile_pool(name=f"c3_{rep}", bufs=1))
    mid = ctx.enter_context(tc.tile_pool(name=f"m3_{rep}", bufs=1))
    # PSUM: 8 banks total. wide mode: 4 x [P,1024] (2 banks each);
    # narrow: 8 x [P,512]. "row" den reserves 2 banks -> shrink the mm ring.
    mm_bufs = (3 if DEN_MODE == "row" else 4) if WIDE_PSUM else \
              (6 if DEN_MODE == "row" else 8)
    psum = ctx.enter_context(tc.tile_pool(
        name=f"p3_{rep}", bufs=mm_bufs, space="PSUM"))

    GW = 2 * CW if WIDE_PSUM else CW  # accumulation-group width

    def _groups(pool, width):
        for c0 in range(0, width, GW):
            cw = min(GW, width - c0)
            yield pool.tile([P, cw], F32, tag="mm", name="mm_ps"), c0, cw
    outp = ctx.enter_context(tc.tile_pool(name=f"o3_{rep}", bufs=2))
    small = ctx.enter_context(tc.tile_pool(name=f"s3_{rep}", bufs=4))
    late = ctx.enter_context(tc.tile_pool(name=f"l3_{rep}", bufs=1))

    ones = const.tile([P, 1], BF16, tag="ones")
    nc.vector.memset(ones, 1.0)

    # PE warm-up during the initial DMA fill (keeps HAM clock-gate warm)
    warm_src = const.tile([P, 512], BF16, tag="warm_src")
    nc.vector.memset(warm_src, 0.0)
    warm_ps = psum.tile([P, 512], F32, tag="mm", name="warm_ps")
    for _ in range(4):
        nc.tensor.matmul(warm_ps, warm_src[:, 0:P], warm_src)

    qt_sb = [mid.tile([P, SH], BF16, tag=f"qt{e}", name=f"qt{e}") for e in range(ET)]
    g_sb = [mid.tile([P, SH], BF16, tag=f"g{d}", name=f"g{d}") for d in range(DT)]
    est_sb = [mid.tile([P, SH], BF16, tag=f"est{j}", name=f"est{j}")
              for j in range(JT)]

    xn_sb = [late.tile([P, D], BF16, tag=f"xn{s}", name=f"xn{s}") for s in range(JT)]
    wv_sb = [late.tile([P, E], BF16, tag=f"wv{d}", name=f"wv{d}") for d in range(DT)]

    with tc.tile_pool(name=f"e3_{rep}", bufs=1) as early:
        xt_sb = [early.tile([P, S], BF16, tag=f"xt{d}", name=f"xt{d}")
                 for d in range(DT)]
        wq_sb = [early.tile([P, E], BF16, tag=f"wq{d}", name=f"wq{d}")
                 for d in range(DT)]
        wkT_sb = [early.tile([P, D], BF16, tag=f"wkT{e}", name=f"wkT{e}")
                  for e in range(ET)]
        # load order tracks first use: QT needs wq + xTr[:, :SH]; G adds wkT;
        # scoresT adds xTr[:, SH:]; AT needs xN; out needs wv last.
        for d in range(DT):
            r = slice(d * P, (d + 1) * P)
            nc.sync.dma_start(wq_sb[d], wq[r, :])
            nc.sync.dma_start(xt_sb[d][:, 0:SH], xTr[r, 0:SH])
        for e in range(ET):
            nc.sync.dma_start(wkT_sb[e], wkT[e * P:(e + 1) * P, :])
        for d in range(DT):
            nc.sync.dma_start(xt_sb[d][:, SH:S], xTr[d * P:(d + 1) * P, SH:S])
        for s in range(JT):
            nc.sync.dma_start(xn_sb[s], xN[s * P:(s + 1) * P, :])
        for d in range(DT):
            nc.sync.dma_start(wv_sb[d], wv[d * P:(d + 1) * P, :])

        # QT[e-block, :] = sum_d wq[d, e-block].T @ xTr[d, :SH]
        for e in range(ET if "qt" in phases else 0):
            for ps, c0, cw in _groups(psum, SH):
                for d in range(DT):
                    for h in range(0, cw, CW):
                        nc.tensor.matmul(ps[:, h:h + CW],
                                         wq_sb[d][:, e * P:(e + 1) * P],
                                         xt_sb[d][:, c0 + h:c0 + h + CW],
                                         start=(d == 0), stop=(d == DT - 1))
                nc.vector.tensor_copy(qt_sb[e][:, c0:c0 + cw], ps)

        # G[d-block, :] = sum_e wkT[e, d-block].T @ QT[e, :]
        for dblk in range(DT if "g" in phases else 0):
            for ps, c0, cw in _groups(psum, SH):
                for e in range(ET):
                    for h in range(0, cw, CW):
                        nc.tensor.matmul(ps[:, h:h + CW],
                                         wkT_sb[e][:, dblk * P:(dblk + 1) * P],
                                         qt_sb[e][:, c0 + h:c0 + h + CW],
                                         start=(e == 0), stop=(e == ET - 1))
                nc.vector.tensor_copy(g_sb[dblk][:, c0:c0 + cw], ps)

        # scores^T[j-block, :] = sum_d xTr[d, j-block].T @ G[d, :]; exp fused
        sctmp = [small.tile([P, SH], F32, tag="sctmp", bufs=2,
                            name=f"sctmp{t}") for t in range(2)] \
            if SC_MODE == "split" else None
        for j in range(JT if "sc" in phases else 0):
            for ps, c0, cw in _groups(psum, SH):
                for d in range(DT):
                    for h in range(0, cw, CW):
                        nc.tensor.matmul(ps[:, h:h + CW],
                                         xt_sb[d][:, j * P:(j + 1) * P],
                                         g_sb[d][:, c0 + h:c0 + h + CW],
                                         start=(d == 0), stop=(d == DT - 1))
                if SC_MODE == "none":  # timing probe: no evacuation
                    pass
                elif SC_MODE == "exp":
                    nc.scalar.activation(est_sb[j][:, c0:c0 + cw], ps,
                                         mybir.ActivationFunctionType.Exp,
                                         scale=float(1.0 / np.sqrt(E)))
                elif SC_MODE == "copy":  # timing probe only: wrong numerics
                    nc.vector.tensor_copy(est_sb[j][:, c0:c0 + cw], ps)
                else:  # "split": DVE evacuates PSUM, ACT exps from SBUF
                    tmp = sctmp[j % 2]
                    nc.vector.tensor_copy(tmp[:, c0:c0 + cw], ps)
                    nc.scalar.activation(est_sb[j][:, c0:c0 + cw],
                                         tmp[:, c0:c0 + cw],
                                         mybir.ActivationFunctionType.Exp,
                                         scale=float(1.0 / np.sqrt(E)))

    # den[q] = sum_s est[s, q]
    recips = None
    recip_t = None
    if "den" in phases and DEN_MODE == "row":
        # ones-lhsT form: den_row[0, q] accumulated over all 16 s-tiles.
        # lhsT is 1 column -> weight load is free; 32 N=512 matmuls total.
        den_ps = psum.tile([P, SH], F32, tag="den", bufs=1, name="den_ps")
        for c0 in range(0, SH, CW):
            for j in range(JT):
                nc.tensor.matmul(den_ps[0:1, c0:c0 + CW], ones,
                                 est_sb[j][:, c0:c0 + CW],
                                 start=(j == 0), stop=(j == JT - 1))
        den_sb = small.tile([1, SH], F32, tag="den_sb", name="den_sb")
        nc.scalar.copy(den_sb, den_ps[0:1, :])
        dram = ctx.enter_context(tc.tile_pool(name=f"dr3_{rep}", bufs=1,
                                              space="DRAM"))
        dr_den = dram.tile([1, SH], F32, tag="dr_den", name="dr_den")
        nc.sync.dma_start(dr_den, den_sb)
        den_t = small.tile([P, IT], F32, tag="den_t", name="den_t")
        nc.sync.dma_start(den_t,
                          dr_den[0, :].rearrange("(i p) -> p i", p=P))
        recip_t = small.tile([P, IT], F32, tag="recip_t", name="recip_t")
        nc.vector.reciprocal(recip_t, den_t)
    elif "den" in phases:
        recips = []
        for i in range(IT):
            den = psum.tile([P, 1], F32, tag="mm", name="den_ps")
            for j in range(JT):
                nc.tensor.matmul(den, est_sb[j][:, i * P:(i + 1) * P], ones,
                                 start=(j == 0), stop=(j == JT - 1))
            recip = small.tile([P, 1], F32, tag="recip", bufs=IT, name="recip")
            nc.vector.reciprocal(recip, den)
            recips.append(recip)

    # AT[d-block, :] = sum_s xN[s, d-block].T @ est[s, :]
    atp = ctx.enter_context(tc.tile_pool(name=f"a3_{rep}", bufs=1))
    at_sb = [atp.tile([P, SH], BF16, tag=f"at{d}", name=f"at{d}")
             for d in range(DT)]
    for dblk in range(DT if "at" in phases else 0):
        for ps, c0, cw in _groups(psum, SH):
            for sj in range(JT):
                for h in range(0, cw, CW):
                    nc.tensor.matmul(ps[:, h:h + CW],
                                     xn_sb[sj][:, dblk * P:(dblk + 1) * P],
                                     est_sb[sj][:, c0 + h:c0 + h + CW],
                                     start=(sj == 0), stop=(sj == JT - 1))
            nc.vector.tensor_copy(at_sb[dblk][:, c0:c0 + cw], ps)

    # out[i-block, :] = (sum_d AT[d, i-block].T @ wv[d, :]) * recip_i
    for i in range(IT if "out" in phases and "den" in phases else 0):
        o = outp.tile([P, E], BF16 if OUT_BF16 else F32, tag="o", name="o_out")
        r_i = recip_t[:, i:i + 1] if recip_t is not None else recips[i]
        for av, c0, cw in _groups(psum, E):
            for d in range(DT):
                for h in range(0, cw, CW):
                    nc.tensor.matmul(av[:, h:h + CW],
                                     at_sb[d][:, i * P:(i + 1) * P],
                                     wv_sb[d][:, c0 + h:c0 + h + CW],
                                     start=(d == 0), stop=(d == DT - 1))
            nc.vector.tensor_scalar_mul(o[:, c0:c0 + cw], av, r_i)
        nc.scalar.dma_start(out[i * P:(i + 1) * P, :], o)


def _build_v3(repeats=1, phases=("qt", "g", "sc", "at", "den", "out")):
    key = ("v3", repeats, tuple(phases), WIDE_PSUM, OUT_BF16, DEN_MODE,
           SC_MODE)
    if key not in _compiled:
        nc = bacc.Bacc("TRN2", target_bir_lowering=False, debug=False,
                       num_devices=8)
        xTr = nc.dram_tensor("xTr", [D, S], BF16, kind="ExternalInput").ap()
        xN = nc.dram_tensor("xN", [S, D], BF16, kind="ExternalInput").ap()
        wq = nc.dram_tensor("wq", [D, E], BF16, kind="ExternalInput").ap()
        wkT = nc.dram_tensor("wkT", [E, D], BF16, kind="ExternalInput").ap()
        wv = nc.dram_tensor("wv", [D, E], BF16, kind="ExternalInput").ap()
        out = nc.dram_tensor("out", [SH, E], BF16 if OUT_BF16 else F32,
                             kind="ExternalOutput").ap()
        with tile.TileContext(nc) as tc:
            for rep in range(repeats):
                with ExitStack() as ctx:
                    _emit_v3(tc, ctx, xTr, xN, wq, wkT, wv, out, rep=rep,
                             phases=phases)
        nc.compile()
        _compiled[key] = nc
    return _compiled[key]


def _make_in_maps_v3(x, wq, wk, wv):
    wq_bf = np.ascontiguousarray(wq).astype(NPBF16)
    wkT_bf = np.ascontiguousarray(np.asarray(wk).T).astype(NPBF16)
    wv_bf = np.ascontiguousarray(wv).astype(NPBF16)
    in_maps = []
    for c in range(8):
        b, h = c // 2, c % 2
        # roll keys so this core's query block is always rows 0:SH
        xr = np.concatenate([x[b, h * SH:(h + 1) * SH], x[b, :h * SH],
                             x[b, (h + 1) * SH:]], axis=0)
        xN = np.ascontiguousarray(xr).astype(NPBF16)
        xTr = np.ascontiguousarray(xr.T).astype(NPBF16)
        in_maps.append({"xTr": xTr, "xN": xN, "wq": wq_bf, "wkT": wkT_bf,
                        "wv": wv_bf})
    return in_maps


def _build_v2(repeats=1):
    key = ("v2", repeats)
    if key not in _compiled:
        nc = bacc.Bacc("TRN2", target_bir_lowering=False, debug=False,
                       num_devices=8)
        xqT = nc.dram_tensor("xqT", [D, SH], BF16, kind="ExternalInput").ap()
        wq = nc.dram_tensor("wq", [D, E], BF16, kind="ExternalInput").ap()
        wk = nc.dram_tensor("wk", [D, E], BF16, kind="ExternalInput").ap()
        wv = nc.dram_tensor("wv", [D, E], BF16, kind="ExternalInput").ap()
        out = nc.dram_tensor("out", [SH, E], F32, kind="ExternalOutput").ap()
        with tile.TileContext(nc) as tc:
            for rep in range(repeats):
                with ExitStack() as ctx:
                    _emit_v2(tc, ctx, xqT, wq, wk, wv, out, rep=rep)
        nc.compile()
        _compiled[key] = nc
    return _compiled[key]


def _make_in_maps_v2(x, wq, wk, wv):
    wq_bf = np.ascontiguousarray(wq).astype(NPBF16)
    wk_bf = np.ascontiguousarray(wk).astype(NPBF16)
    wv_bf = np.ascontiguousarray(wv).astype(NPBF16)
    in_maps = []
    for c in range(8):
        b, h = c // 2, c % 2
        xqT = np.ascontiguousarray(x[b, h * SH:(h + 1) * SH].T).astype(NPBF16)
        in_maps.append({"xqT": xqT, "wq": wq_bf, "wk": wk_bf, "wv": wv_bf})
    return in_maps


def _build(repeats=1, phases=("proj", "scores", "av")):
    key = (repeats, tuple(phases), NARROW_PSUM, EARLY_V)
    if key not in _compiled:
        nc = bacc.Bacc("TRN2", target_bir_lowering=False, debug=False,
                       num_devices=8)
        xTr = nc.dram_tensor("xTr", [D, S], BF16, kind="ExternalInput").ap()
        wq = nc.dram_tensor("wq", [D, E], BF16, kind="ExternalInput").ap()
        wk = nc.dram_tensor("wk", [D, E], BF16, kind="ExternalInput").ap()
        wv = nc.dram_tensor("wv", [D, E], BF16, kind="ExternalInput").ap()
        out = nc.dram_tensor("out", [SH, E], F32, kind="ExternalOutput").ap()
        with tile.TileContext(nc) as tc:
            for rep in range(repeats):
                with ExitStack() as ctx:
                    _emit(tc, ctx, xTr, wq, wk, wv, out, rep=rep, phases=phases)
        nc.compile()
        _compiled[key] = nc
    return _compiled[key]


def _make_in_maps(x, wq, wk, wv):
    wq_bf = np.ascontiguousarray(wq).astype(NPBF16)
    wk_bf = np.ascontiguousarray(wk).astype(NPBF16)
    wv_bf = np.ascontiguousarray(wv).astype(NPBF16)
    in_maps = []
    for c in range(8):
        b, h = c // 2, c % 2
        # roll keys so this core's query block is always columns 0:SH
        xr = np.concatenate([x[b, h * SH:(h + 1) * SH], x[b, :h * SH],
                             x[b, (h + 1) * SH:]], axis=0)
        xTr = np.ascontiguousarray(xr.T).astype(NPBF16)
        in_maps.append({"xTr": xTr, "wq": wq_bf, "wk": wk_bf, "wv": wv_bf})
    return in_maps


VERSION = 3


def _build_any(repeats=1):
    if VERSION == 3:
        return _build_v3(repeats)
    return _build_v2(repeats) if VERSION == 2 else _build(repeats)


def _make_maps_any(x, wq, wk, wv):
    mk = {1: _make_in_maps, 2: _make_in_maps_v2, 3: _make_in_maps_v3}[VERSION]
    return mk(np.asarray(x, np.float32), np.asarray(wq), np.asarray(wk),
              np.asarray(wv))


def kernel(x, wq, wk, wv, _trace=False):
    x = np.asarray(x, dtype=np.float32)
    nc = _build_any()
    in_maps = _make_maps_any(x, wq, wk, wv)
    try:
        res = run_bass_kernel_spmd(nc, in_maps, core_ids=list(range(8)),
                                   trace=_trace)
    except Exception:
        # transient NRT_EXEC_UNIT_UNRECOVERABLE wedges have been observed to
        # clear on a fresh attempt
        time_mod.sleep(5)
        res = run_bass_kernel_spmd(nc, in_maps, core_ids=list(range(8)),
                                   trace=_trace)
    full = np.empty((B, S, E), np.float32)
    for c in range(8):
        b, h = c // 2, c % 2
        full[b, h * SH:(h + 1) * SH] = np.asarray(
            res.results[c]["out"], dtype=np.float32)
    if _trace:
        kernel.last_results = res
    return full

